# revision 1
# baseline (speedup 1.0000x reference)
"""Trainium2 Bass kernel for NarrativeClassificationLoss.

Data-parallel over batch: each of 8 cores processes a 2048-row shard and
emits per-class partial sums; the host combines them in float64 (the
pos_weight "all-reduce" over the batch happens at gather time).

Per-element math (partition dim = batch rows, all intermediates bf16):
  E    = exp(-x)                 [ACT, natural_log_exp table set]
  spn  = softplus(-x) = ln(E+1)  [ACT]
  spp  = softplus(x)  = x + spn  [DVE]
  u    = y * spn                 [DVE]
  rn   = (1-y) * spp             [DVE tensor_scalar + tensor_tensor]
  e    = exp(-2*spp) = (1-sigmoid(x))^2   [ACT]
Per-class reductions over the batch go through TensorE into PSUM:
  C_full[n,c] = sum_b nl[b,n] * u[b,c]     (lhsT = narrative labels;
  D_full[n,c] = sum_b nl[b,n] * rn[b,c]     host reads diagonal blocks)
  S_s[c]      = sum_b s[b,c]               (lhsT = ones)
  A|Bn|S_n    = ones-reduce of packed [u_n | rn_n | nl] (single matmul
                group per PSUM bank: a start=True wipes the whole bank)
  FC         += e_blk^T @ u_blk            (shared focal Frobenius
                accumulator; narrative part pre-scaled x8 via
                exp(..+ln8) so tr(FC) carries both focal terms with the
                common 1/(B*1024) weight)
Hierarchy: sigmoid(max_k x_sub) = exp(-min_k softplus(-x_sub)), so the
group reduction is a pairwise MIN tree over the already-computed spn
(no extra transcendentals); relu+row-accumulate on DVE tensor_scalar.
"""

import numpy as np

import concourse.bacc as bacc
import concourse.tile as tile
from concourse import mybir
from concourse.bass_utils import run_bass_kernel_spmd

B = 16384
NCORES = 8
BL = B // NCORES          # 2048 rows per core
NN = 128                  # narrative classes
NS = 1024                 # subnarrative classes
K = NS // NN              # 8 subnarratives per narrative
NT = BL // 128            # 16 batch tiles of 128 rows
NM = NT // 2              # 8 mega-tiles of 256 rows

f32 = mybir.dt.float32
bf16 = mybir.dt.bfloat16
i32 = mybir.dt.int32
AF = mybir.ActivationFunctionType
OP = mybir.AluOpType
LN8 = 2.0794415416798357

_CACHE = {}
MEGAS = [(0, 1), (1, 2), (3, 2), (5, 2), (7, 1), (8, 2), (10, 2), (12, 2), (14, 1), (15, 1)]
ACT_CHAIN_ON = True
LAST_RESULT = None

_ACT_SET = "natural_log_exp_and_others"


def _pin_act_tables(nc):
    """Thin the activation-table map so every func we use resolves to the
    one set that contains exp+ln+copy+relu — a single table load instead
    of thrashing between exp_and_others and natural_log_exp_and_others."""
    from concourse.hw_specs import get_activation_tables

    tabs = get_activation_tables(nc.m.arch)  # functools.cache'd: mutate in place
    ours = {AF.Exp, AF.Ln, AF.Copy, AF.Relu, AF.Identity}
    if _ACT_SET in tabs and ours - {AF.Identity} <= tabs[_ACT_SET]:
        for name, s in tabs.items():
            if name != _ACT_SET:
                s -= ours


def _build(reps=1):
    nc = bacc.Bacc()
    _pin_act_tables(nc)

    xn = nc.declare_dram_parameter("narrative_logits", [BL, NN], f32, isOutput=False)
    xs = nc.declare_dram_parameter("subnarrative_logits", [BL, NS], f32, isOutput=False)
    yn = nc.declare_dram_parameter("narrative_labels", [BL, NN], i32, isOutput=False)
    ys = nc.declare_dram_parameter("subnarrative_labels", [BL, NS], i32, isOutput=False)

    o_c = nc.declare_dram_parameter("o_c", [NN, NS], bf16, isOutput=True)
    o_d = nc.declare_dram_parameter("o_d", [NN, NS], bf16, isOutput=True)
    o_f = nc.declare_dram_parameter("o_f", [NN, NN], f32, isOutput=True)
    o_abs = nc.declare_dram_parameter("o_abs", [1, 512], f32, isOutput=True)
    o_ss = nc.declare_dram_parameter("o_ss", [1, NS], f32, isOutput=True)
    o_h = nc.declare_dram_parameter("o_h", [NN, 4], f32, isOutput=True)

    with tile.TileContext(nc) as tc:
        with (
            tc.tile_pool(name="persist", bufs=1) as P1,
            tc.tile_pool(name="stream", bufs=2) as ST,
            tc.tile_pool(name="psum", bufs=1, space="PSUM") as PS,
        ):
            ones = P1.tile([128, 1], bf16)
            nc.vector.memset(ones, 1.0)
            ln8 = P1.tile([128, 1], f32)
            nc.vector.memset(ln8, LN8)

            for _rep in range(reps):
              _emit(nc, P1, ST, PS, ones, ln8, xn, xs, yn, ys,
                    o_c, o_d, o_f, o_abs, o_ss, o_h)

    nc.finalize()
    return nc


def _emit(nc, P1, ST, PS, ones, ln8, xn, xs, yn, ys, o_c, o_d, o_f, o_abs, o_ss, o_h):
    # persistent slabs (see file docstring for the math)
    ubn = P1.tile([128, NT, 384], bf16)
    u_n = ubn[:, :, 0:128]
    rn_n = ubn[:, :, 128:256]
    nl_f = ubn[:, :, 256:384]                # narrative labels 0/1
    p_n = P1.tile([128, NT, NN], bf16)       # sigmoid(narr logits)
    # group-MIN of softplus(-x_sub): softplus(-max_k x) = min_k softplus(-x_k)
    sm_all = P1.tile([128, NT, NN], bf16)
    e8_n = P1.tile([128, NT, NN], bf16)
    hacc = P1.tile([128, 4], f32)

    # PSUM accumulators (8 banks exactly)
    C0 = PS.tile([128, 512], f32, tag="C0")
    C1 = PS.tile([128, 512], f32, tag="C1")
    D0 = PS.tile([128, 512], f32, tag="D0")
    D1 = PS.tile([128, 512], f32, tag="D1")
    S0 = PS.tile([1, 512], f32, tag="S0")
    S1 = PS.tile([1, 512], f32, tag="S1")
    AB = PS.tile([1, 512], f32, tag="AB")
    FC = PS.tile([128, 128], f32, tag="FC")

    xsr = xs[:, :].rearrange("(m q p) c -> m p q c", q=16, p=128)[0]  # [128,16,1024]
    ysr = ys[:, :].rearrange("(m q p) c -> m p q c", q=16, p=128)[0]
    xnr = xn[:, :].rearrange("(t p) c -> p t c", p=128)
    ynr = yn[:, :].rearrange("(t p) c -> p t c", p=128)

    # mega-tile schedule: (start_t, n_tiles); last mega split for a shorter drain
    megas = MEGAS

    def load_mega(t0, nt):
        xt = ST.tile([128, nt, NS], f32, tag="xs", bufs=3)
        lt = ST.tile([128, nt, NS], i32, tag="ls", bufs=3)
        nc.sync.dma_start(out=xt, in_=xsr[:, t0 : t0 + nt, :])
        nc.sync.dma_start(out=lt, in_=ysr[:, t0 : t0 + nt, :])
        return xt, lt

    def compute_mega(t0, nt, xt, lt):
        sf = ST.tile([128, nt, NS], bf16, tag="sf")
        nc.gpsimd.tensor_copy(out=sf, in_=lt)
        xb = ST.tile([128, nt, NS], bf16, tag="xb")
        nc.gpsimd.tensor_copy(out=xb, in_=xt)

        Et = ST.tile([128, nt, NS], bf16, tag="Et")
        nc.scalar.activation(Et, xt, AF.Exp, scale=-1.0)
        spn = ST.tile([128, nt, NS], bf16, tag="spn", bufs=3)
        nc.scalar.activation(spn, Et, AF.Ln, bias=1.0)

        u = ST.tile([128, nt, NS], bf16, tag="u")
        nc.vector.tensor_mul(u, sf, spn)
        spp = ST.tile([128, nt, NS], bf16, tag="spp", bufs=3)
        nc.vector.tensor_add(spp, xb, spn)
        ns = ST.tile([128, nt, NS], bf16, tag="nsg")
        nc.vector.tensor_scalar(ns, sf, -1.0, 1.0, op0=OP.mult, op1=OP.add)
        rn = ST.tile([128, nt, NS], bf16, tag="rn")
        nc.vector.tensor_mul(rn, ns, spp)
        e = ST.tile([128, nt, NS], bf16, tag="e", bufs=3)
        nc.scalar.activation(e, spp, AF.Exp, scale=-2.0)

        # grouped MIN of spn over K=8 via pairwise tree
        sg = spn.rearrange("p q (n k) -> p q n k", k=K)
        m1 = ST.tile([128, nt, NN, 4], bf16, tag="m1", bufs=1)
        nc.vector.tensor_tensor(m1, sg[:, :, :, 0:4], sg[:, :, :, 4:8], op=OP.min)
        m2 = ST.tile([128, nt, NN, 2], bf16, tag="m2", bufs=1)
        nc.vector.tensor_tensor(m2, m1[:, :, :, 0:2], m1[:, :, :, 2:4], op=OP.min)
        nc.vector.tensor_tensor(
            sm_all[:, t0 : t0 + nt, :], m2[:, :, :, 0], m2[:, :, :, 1], op=OP.min
        )

        for q in range(nt):
            t = t0 + q
            st = t == 0
            sp = t == NT - 1
            nlT = nl_f[:, t, :]
            nc.tensor.matmul(C0, nlT, u[:, q, 0:512], start=st, stop=sp)
            nc.tensor.matmul(C1, nlT, u[:, q, 512:1024], start=st, stop=sp)
            nc.tensor.matmul(D0, nlT, rn[:, q, 0:512], start=st, stop=sp)
            nc.tensor.matmul(D1, nlT, rn[:, q, 512:1024], start=st, stop=sp)
            nc.tensor.matmul(S0, ones, sf[:, q, 0:512], start=st, stop=sp)
            nc.tensor.matmul(S1, ones, sf[:, q, 512:1024], start=st, stop=sp)
            for j in range(K):
                nc.tensor.matmul(
                    FC,
                    e[:, q, j * 128 : (j + 1) * 128],
                    u[:, q, j * 128 : (j + 1) * 128],
                    start=False,
                    stop=(t == NT - 1 and j == K - 1),
                )

    def hier_chunk(ci, h0, hn):
        pg = ST.tile([128, hn, NN], bf16, tag="pg", bufs=1)
        nc.scalar.activation(pg, sm_all[:, h0 : h0 + hn, :], AF.Exp, scale=-1.0)
        gd = ST.tile([128, hn, NN], bf16, tag="gd", bufs=1)
        nc.vector.tensor_sub(gd, pg, p_n[:, h0 : h0 + hn, :])
        hm = ST.tile([128, hn, NN], bf16, tag="hm", bufs=1)
        nc.vector.tensor_mul(hm, gd, nl_f[:, h0 : h0 + hn, :])
        hr = ST.tile([128, hn, NN], bf16, tag="hr", bufs=1)
        nc.vector.tensor_scalar(hr, hm, 0.0, 0.0, op0=OP.max, op1=OP.add,
                                accum_out=hacc[:, ci : ci + 1])

    # ---- emission order tuned for pipeline fill:
    # mega0 DMA first (feeds ACT earliest), then narrative DMA + casts,
    # mega0/1 compute, then the narrative chain, then the rest.
    xn_sb = ST.tile([128, NT, NN], f32, tag="xs", bufs=3)
    yn_sb = ST.tile([128, NT, NN], i32, tag="ls", bufs=3)
    nc.sync.dma_start(out=xn_sb, in_=xnr)
    nc.sync.dma_start(out=yn_sb, in_=ynr)
    xnb = ST.tile([128, NT, NN], bf16, tag="xb")
    nc.vector.tensor_copy(out=nl_f, in_=yn_sb)
    nc.vector.tensor_copy(out=xnb, in_=xn_sb)

    # ---------------- narrative chain ----------------
    E_n = ST.tile([128, NT, NN], bf16, tag="Et")
    nc.scalar.activation(E_n, xn_sb, AF.Exp, scale=-1.0)
    spn_n = ST.tile([128, NT, NN], bf16, tag="spn", bufs=3)
    nc.scalar.activation(spn_n, E_n, AF.Ln, bias=1.0)
    nc.scalar.activation(p_n, spn_n, AF.Exp, scale=-1.0)

    spp_n = ST.tile([128, NT, NN], bf16, tag="spp", bufs=3)
    nc.vector.tensor_add(spp_n, xnb, spn_n)
    nc.scalar.activation(e8_n, spp_n, AF.Exp, scale=-2.0, bias=ln8)

    nc.vector.tensor_mul(u_n, nl_f, spn_n)
    ns_n = ST.tile([128, NT, NN], bf16, tag="nsg")
    nc.vector.tensor_scalar(ns_n, nl_f, -1.0, 1.0, op0=OP.mult, op1=OP.add)
    nc.vector.tensor_mul(rn_n, ns_n, spp_n)

    for t in range(NT):
        nc.tensor.matmul(AB[:, 0:384], ones, ubn[:, t, :],
                         start=(t == 0), stop=(t == NT - 1))
        nc.tensor.matmul(FC, e8_n[:, t, :], u_n[:, t, :],
                         start=(t == 0), stop=False)

    # ---------------- remaining stream ----------------
    for t0, nt in megas:
        xt, lt = load_mega(t0, nt)
        compute_mega(t0, nt, xt, lt)
        if t0 + nt == 8:
            hier_chunk(0, 0, 8)
        elif t0 + nt == 15:
            hier_chunk(1, 8, 7)
    hier_chunk(2, 15, 1)

    # ---------------- evacuate + store ----------------
    C_sb = P1.tile([128, NS], bf16)
    nc.scalar.copy(C_sb[:, 0:512], C0)
    nc.scalar.copy(C_sb[:, 512:1024], C1)
    D_sb = P1.tile([128, NS], bf16)
    nc.scalar.copy(D_sb[:, 0:512], D0)
    nc.scalar.copy(D_sb[:, 512:1024], D1)
    F_sb = P1.tile([128, NN], f32)
    nc.vector.tensor_copy(F_sb, FC)
    AB_sb = P1.tile([1, 512], f32)
    nc.vector.tensor_copy(AB_sb, AB)
    Ss_sb = P1.tile([1, NS], f32)
    nc.vector.tensor_copy(Ss_sb[:, 0:512], S0)
    nc.vector.tensor_copy(Ss_sb[:, 512:1024], S1)

    nc.sync.dma_start(out=o_c[:, :], in_=C_sb)
    nc.sync.dma_start(out=o_d[:, :], in_=D_sb)
    nc.sync.dma_start(out=o_f[:, :], in_=F_sb)
    nc.sync.dma_start(out=o_abs[:, :], in_=AB_sb)
    nc.sync.dma_start(out=o_ss[:, :], in_=Ss_sb)
    nc.sync.dma_start(out=o_h[:, :], in_=hacc)


def kernel(
    narrative_logits, subnarrative_logits, narrative_labels, subnarrative_labels
):
    global LAST_RESULT
    if "nc" not in _CACHE:
        _CACHE["nc"] = _build()
    nc = _CACHE["nc"]

    in_maps = []
    for i in range(NCORES):
        s = slice(i * BL, (i + 1) * BL)
        in_maps.append(
            {
                "narrative_logits": np.ascontiguousarray(narrative_logits[s]),
                "subnarrative_logits": np.ascontiguousarray(subnarrative_logits[s]),
                "narrative_labels": np.ascontiguousarray(narrative_labels[s]),
                "subnarrative_labels": np.ascontiguousarray(subnarrative_labels[s]),
            }
        )

    res = run_bass_kernel_spmd(nc, in_maps, list(range(NCORES)))
    LAST_RESULT = res

    # ------- host combine (the batch "all-reduce") -------
    Af = np.zeros(NN, np.float64)
    Bneg = np.zeros(NN, np.float64)
    Sn = np.zeros(NN, np.float64)
    Ss = np.zeros(NS, np.float64)
    Cf = np.zeros((NN, NS), np.float64)
    Df = np.zeros((NN, NS), np.float64)
    Ff = np.zeros((NN, NN), np.float64)
    H = 0.0
    for r in res.results:
        ab = r["o_abs"][0].astype(np.float64)
        Af += ab[0:128]
        Bneg += ab[128:256]
        Sn += ab[256:384]
        Ss += r["o_ss"][0].astype(np.float64)
        Cf += r["o_c"].astype(np.float64)
        Df += r["o_d"].astype(np.float64)
        Ff += r["o_f"].astype(np.float64)
        H += r["o_h"].astype(np.float64).sum()

    cc = np.arange(NS)
    Cd = Cf[cc // K, cc]
    Dd = Df[cc // K, cc]
    Bn = Bneg

    npw = np.clip((B - Sn) / (Sn + 1e-6), 1.0, 50.0)
    spw = np.clip((B - Ss) / (Ss + 1e-6), 1.0, 50.0)

    narrative_loss = (npw * Af + Bn).sum() / (B * NN)

    valid = Sn.sum()
    sub_loss = (spw * Cd + Dd).sum() / K / max(valid, 1.0) if valid > 0 else 0.0

    hier = H / B
    focal = 0.1 * np.trace(Ff) / (B * NS)

    total = narrative_loss + sub_loss + 0.5 * hier + focal
    return np.asarray(total, dtype=np.float32)

